# revision 28
# baseline (speedup 1.0000x reference)
"""AdSBHNet integral kernel for 8 TRN2 NeuronCores — transposed layout.

Math (all-real reformulation of the complex reference):
  poly(c,z) = sum_{i=1..5} c_i z^i ;  f = (1-z^4) e^{poly(a,z)} ; g = e^{poly(b,z)}/(1-z^4)
  z = zs*u.
  L: w  = A/(D+eps+i*eps) - 1 + eps(1+i),  A = zs^4 f(z), D = z^4 f(zs)
     integrand = sqrt(g)/sqrt(w);  L = (2/pi) * zs * sum_j(w_j * integrand_j)
  V: inner = 1 - Y/(X+eps+i*eps) + eps(1+i), Y = z^4 f(zs), X = zs^4 f(z)
     term = sqrt(f g)/sqrt(inner) - 1; integrand = term/(z^2+eps(1+i))
     V = 2pi*zs*sum_j(w_j integrand_j) - 2pi/zs
  Complex sqrt of w=re+i*im with r=|w|: sqrt(w) = p + i*q, p=sqrt((r+re)/2),
  q=sign(im)*sqrt((r-re)/2); 1/sqrt(w) = (p-i*q)/r.  For sqrt(g)/sqrt(w):
    sqrt(g)*p/r = sqrt(G*(r+re)), sqrt(g)*|q|/r = sqrt(G*(r-re)), G = g/(2 r^2).
  r-|re| cancels catastrophically, so compute rlarge = r+|re| and
  rsmall = im^2/rlarge and route by sign(re) with predicated copies.

Quadrature: the reference's 2000/1500-pt uniform trapezoid sums are replaced
by an equivalent 128-node mixed rule per integral (validated ~6e-6 relative):
Gauss-Legendre on the smooth middle + the exact trapezoid points near the
eps-regularized singular ends (L: 64 GL + last 64 pts; V: first 48 pts +
48 GL + last 32 pts).

Layout: u-nodes live in the PARTITION dim (128 exactly), the 1024 batch rows
of each core in the FREE dim. Per-node constants are [128,1] per-partition
scalars (free in tensor_scalar/bias); per-row quantities are [1,1024] rows
broadcast once via a K=1 PE matmul. Each integral is ONE wide pass of
~30 instructions of free-size 1024. poly(a,z)+ln(zs^4) comes from a K=6
TensorE matmul (lhsT = const u-powers [6,128], rhs = a_k-scaled zs-powers
[6,1024], bias rows ride as k=0 terms). The weighted node-sum is a K=128 PE
matmul with the quadrature weights (x final scale) in lhsT.

Sharding: pure data parallel, zs batch split 8 ways; a/b replicated.
"""

import math
import sys

import numpy as np

sys.path.insert(0, "/opt/trn_rl_repo")

import concourse.bass as bass
import concourse.bacc as bacc
import concourse.mybir as mybir
from concourse import bass_utils
from concourse.tile import TileContext

F32 = mybir.dt.float32
I32 = mybir.dt.int32
BF16 = mybir.dt.bfloat16
U16 = mybir.dt.uint16
OP = mybir.AluOpType
AF = mybir.ActivationFunctionType

EPS = 1e-6
EPS2 = EPS * EPS
NU_L = 2000
NU_V = 1500
B = 8192
NCORES = 8
BLOC = B // NCORES       # 1024 rows per core
H_L = (1.0 - 2 * EPS) / (NU_L - 1)
H_V = (1.0 - 2 * EPS) / (NU_V - 1)
LN2 = math.log(2.0)
NN = 128                 # u-nodes per integral == partition count


def _mixed_nodes(Nu, h, n_head, n_gl, n_tail):
    """Nodes/weights replicating the Nu-pt trapezoid sum h*(f0/2+...+fN/2)
    with Gauss-Legendre on the smooth middle (float64)."""
    u = EPS + h * np.arange(Nu)
    nodes, wts = [], []
    if n_head:
        nodes.append(u[: n_head + 1])
        w = np.full(n_head + 1, h)
        w[0] = w[-1] = h / 2
        wts.append(w)
    lo = u[n_head]
    hi = u[Nu - 1 - n_tail]
    x, w = np.polynomial.legendre.leggauss(n_gl)
    nodes.append(0.5 * (hi + lo) + 0.5 * (hi - lo) * x)
    wts.append(0.5 * (hi - lo) * w)
    nodes.append(u[Nu - 1 - n_tail:])
    wt = np.full(n_tail + 1, h)
    wt[0] = wt[-1] = h / 2
    wts.append(wt)
    return np.concatenate(nodes), np.concatenate(wts)


_UL, _WL = _mixed_nodes(NU_L, H_L, 0, 64, 63)      # 128 nodes
_UV, _WV = _mixed_nodes(NU_V, H_V, 47, 48, 31)     # 128 nodes
assert len(_UL) == NN and len(_UV) == NN

_K6 = np.arange(6.0)
_UPOWL_T = (_UL[None, :] ** _K6[:, None]).astype(np.float32)   # [6, 128]
_UPOWV_T = (_UV[None, :] ** _K6[:, None]).astype(np.float32)   # [6, 128]
# per-node columns: u4L, -u4L, u4V, -u4V, u2V,
#                   wLre=(2/pi)wL, wLim=-(2/pi)wL, wVre=2pi wV, wVim=-2pi wV
_NCOL = np.stack(
    [
        _UL**4, -(_UL**4), _UV**4, -(_UV**4), _UV**2,
        (2.0 / math.pi) * _WL, -(2.0 / math.pi) * _WL,
        2.0 * math.pi * _WV, -2.0 * math.pi * _WV,
    ],
    axis=1,
).astype(np.float32)                                            # [128, 9]
_NCOL_BF = _NCOL[:, 5:9].astype(np.float32)  # weight cols, bf16-cast on chip

# ---- custom DVE ops (registered into concourse.dve_ops at import) ---------
import concourse.dve_ops as _dops
from concourse.dve_spec import C0 as _C0
from concourse.dve_spec import C1 as _C1
from concourse.dve_spec import C2 as _C2
from concourse.dve_spec import Spec as _Spec
from concourse.dve_spec import Src0 as _Src0
from concourse.dve_spec import Src1 as _Src1
from concourse.dve_spec import _has_src1 as _hs1
from concourse.dve_spec import lower as _dve_lower
from concourse.dve_spec import sq as _sq
from concourse.dve_uop import DveOpSpec as _DveOpSpec


def _register_dve(name, spec):
    for op in _dops.OPS:
        if op.name == name:
            return op
    row = _dops._CUSTOM_DVE_ROW_BASE + len(_dops.OPS)
    assert row < 0x20
    _dops._SUB_OPCODE_FOR_NAME[name] = row
    shas = {}
    for ver in ("v3", "v4"):
        tmp = _DveOpSpec(name=name, opcode=row, uops=_dve_lower(spec, ver=ver),
                         rd1_en=_hs1(spec))
        shas[ver] = tmp.sha(ver)
    op = _dops.DveOp(name, spec, subdim=False, uops_sha=shas)
    _dops.OPS.append(op)
    return op


# out = (in0*s0 + s1)^2 + imm2   (n2 = (c1*u4+eps)^2+eps^2 etc.)
_AFFSQ = _register_dve("ANT_AFFSQ", _Spec(
    body=_sq(_Src0 * _C0 + _C1) + _C2,
    reference=lambda in0, in1, s0, s1, imm2: (in0 * s0 + s1) ** 2 + imm2,
))
# out = in0 * (in1*s0 + s1)      (tDp = t*(c1*u4+eps) etc.)
_MULAFF = _register_dve("ANT_MULAFF", _Spec(
    body=_Src0 * (_Src1 * _C0 + _C1),
    reference=lambda in0, in1, s0, s1, imm2: in0 * (in1 * s0 + s1),
))
from concourse.dve_spec import Zero as _Zero
from concourse.dve_spec import maxx as _maxx
from concourse.dve_spec import select as _select

# out = (in0-s0)^2 + in1        (r2s = re^2 + im^2)
_SQD_ADD = _register_dve("ANT_SQDADD", _Spec(
    body=_sq(_Src0 - _C0) + _Src1,
    reference=lambda in0, in1, s0, s1, imm2: (in0 - s0) ** 2 + in1,
))
from concourse.dve_spec import One as _One1

# out = (in0-s0)^2 + imm2*(1-in1)^2   (r2 = re^2 + im^2 from tDp,t)
_R2FULL = _register_dve("ANT_R2FULL", _Spec(
    body=_sq(_Src0 - _C0) + _sq(_One1 - _Src1) * _C2,
    reference=lambda in0, in1, s0, s1, imm2: (in0 - s0) ** 2 + imm2 * (1.0 - in1) ** 2,
))
# out = (in0-s0)^2 + imm2*(1+in1)^2   (V variant: im2 = eps^2 (1+t2)^2)
_R2FULLP = _register_dve("ANT_R2FULLP", _Spec(
    body=_sq(_Src0 - _C0) + _sq(_One1 + _Src1) * _C2,
    reference=lambda in0, in1, s0, s1, imm2: (in0 - s0) ** 2 + imm2 * (1.0 + in1) ** 2,
))
# out = |in0-s0| + in1          (rlg = |re| + r)
_ABSD_ADD = _register_dve("ANT_ABSDADD", _Spec(
    body=_maxx(_Src0 - _C0, _C0 - _Src0) + _Src1,
    reference=lambda in0, in1, s0, s1, imm2: np.abs(in0 - s0) + in1,
))
# out = in0 * sign-ish(s0 - in1): +in0 where in1 <= s0 else -in0
_SGN_LE = _register_dve("ANT_SGNLE", _Spec(
    body=_select(_Src1 <= _C0, _Src0, _Zero - _Src0),
    reference=lambda in0, in1, s0, s1, imm2: np.where(in1 <= s0, in0, -in0),
))
from concourse.dve_spec import One as _One

# out = in1*(1-in1)*in0          (c1 = zs4*(1-zs4)*e^pa(zs))
_C1ROW = _register_dve("ANT_C1ROW", _Spec(
    body=_Src1 * (_One - _Src1) * _Src0,
    reference=lambda in0, in1, s0, s1, imm2: in1 * (1.0 - in1) * in0,
))


def build_nc(reps=1):
    nc = bacc.Bacc("TRN2", target_bir_lowering=False, debug=False, num_devices=NCORES)
    a_d = nc.declare_dram_parameter("a", [5], F32, isOutput=False)
    b_d = nc.declare_dram_parameter("b", [5], F32, isOutput=False)
    zs_d = nc.declare_dram_parameter("zs", [BLOC], F32, isOutput=False)
    out_d = nc.declare_dram_parameter("out", [4, BLOC], F32, isOutput=True)

    upowL_d = nc.inline_tensor(_UPOWL_T, name="upowL")
    upowV_d = nc.inline_tensor(_UPOWV_T, name="upowV")
    ncol_d = nc.inline_tensor(_NCOL, name="ncol")

    with TileContext(nc) as tc:
        with (
            tc.tile_pool(name="cst", bufs=1) as cst,
            tc.tile_pool(name="wk", bufs=1) as wk,
            tc.tile_pool(name="ps", bufs=1, space="PSUM") as pspool,
            tc.tile_pool(name="pss", bufs=1, space="PSUM") as pssetup,
        ):
            v = nc.vector
            sc = nc.scalar
            gp = nc.gpsimd

            def W(tag, dt=F32, nm=None):
                return wk.tile([NN, BLOC], dt, tag=tag, name=nm or f"t{tag}")

            def R(tag, dt=F32, nm=None):
                return wk.tile([1, BLOC], dt, tag=tag, name=nm or f"r{tag}")

            # ---------------- setup ----------------
            zrow = cst.tile([1, BLOC], F32)
            for _c in range(0, BLOC, 512):
                nc.sync.dma_start(
                    out=zrow[:, _c:_c + 512],
                    in_=zs_d[_c:_c + 512].rearrange("(o n) -> o n", o=1))
            upL = cst.tile([6, NN], F32)
            nc.sync.dma_start(out=upL[:], in_=upowL_d[:, :])
            upV = cst.tile([6, NN], F32)
            nc.sync.dma_start(out=upV[:], in_=upowV_d[:, :])
            ncol = cst.tile([NN, 9], F32)
            nc.sync.dma_start(out=ncol[:], in_=ncol_d[:, :])
            wcols = cst.tile([NN, 4], BF16)
            v.tensor_copy(wcols[:], ncol[:, 5:9])

            aext = cst.tile([6, 1], F32)
            bext = cst.tile([6, 1], F32)
            v.memset(aext[:], 0.0)
            v.memset(bext[:], 0.0)
            nc.sync.dma_start(out=aext[1:6, 0:1], in_=a_d[:])
            nc.sync.dma_start(out=bext[1:6, 0:1], in_=b_d[:])
            abext = cst.tile([6, 1], F32)
            v.tensor_tensor(abext[:], aext[:], bext[:], OP.add)

            ones6 = cst.tile([1, 6], F32)
            v.memset(ones6[:], 1.0)
            ones128 = cst.tile([1, NN], F32)
            v.memset(ones128[:], 1.0)
            kcol_i = cst.tile([6, 1], I32)
            gp.iota(kcol_i[:], pattern=[[1, 1]], base=0, channel_multiplier=1)
            kcol6 = cst.tile([6, 1], F32)
            v.tensor_copy(kcol6[:], kcol_i[:])

            lnz = cst.tile([1, BLOC], F32)
            sc.activation(lnz[:], zrow[:], AF.Ln)

            # zpow [6, BLOC]: row k = zs^k via exp(k ln zs)
            klnz = wk.tile([6, BLOC], F32, tag="s0", name="klnz")
            for c0 in range(0, BLOC, 512):
                ps6 = pssetup.tile([6, 512], F32, tag="pd", name=f"ps6_{c0}")
                nc.tensor.matmul(ps6[:], ones6[:], lnz[:, c0:c0 + 512],
                                 start=True, stop=True)
                v.tensor_scalar(klnz[:, c0:c0 + 512], ps6[:], kcol6[:], None, OP.mult)
            zpow = cst.tile([6, BLOC], F32)
            sc.activation(zpow[:], klnz[:], AF.Exp)

            # matmul rhs tensors [6, BLOC]
            lnzs4row = cst.tile([1, BLOC], F32)
            gp.tensor_scalar(lnzs4row[:], lnz[:], 4.0, None, OP.mult)
            rhs_pa = cst.tile([6, BLOC], F32)
            gp.tensor_scalar(rhs_pa[:], zpow[:], aext[:], None, OP.mult)
            sc.activation(rhs_pa[0:1, :], lnzs4row[:], AF.Copy)
            rhs_pb = cst.tile([6, BLOC], F32)
            gp.tensor_scalar(rhs_pb[:], zpow[:], bext[:], None, OP.mult)
            sc.activation(rhs_pb[0:1, :], lnz[:], AF.Copy, scale=2.0)
            rhs_pab = cst.tile([6, BLOC], F32)
            gp.tensor_scalar(rhs_pab[:], zpow[:], abext[:], None, OP.mult)

            # per-row quantities [1, BLOC]
            zs2row = cst.tile([1, BLOC], F32)
            v.tensor_tensor(zs2row[:], zrow[:], zrow[:], OP.mult)
            zs4row = cst.tile([1, BLOC], F32)
            v.tensor_tensor(zs4row[:], zs2row[:], zs2row[:], OP.mult)
            # pa(zs): K=6 matmul -> [1, BLOC]
            e_pazrow = cst.tile([1, BLOC], F32)
            for c0 in range(0, BLOC, 512):
                pz = pssetup.tile([1, 512], F32, tag="pd", name=f"pz_{c0}")
                nc.tensor.matmul(pz[:], aext[:], zpow[:, c0:c0 + 512],
                                 start=True, stop=True)
                sc.activation(e_pazrow[:, c0:c0 + 512], pz[:], AF.Exp)
            omzs4row = R("s1", nm="omzs4row")
            gp.tensor_scalar(omzs4row[:], zs4row[:], -1.0, 1.0, OP.mult, OP.add)
            fzsrow = R("s2", nm="fzsrow")
            gp.tensor_tensor(fzsrow[:], e_pazrow[:], omzs4row[:], OP.mult)
            c1row = cst.tile([1, BLOC], F32)
            v.tensor_tensor(c1row[:], zs4row[:], fzsrow[:], OP.mult)
            invzrow = cst.tile([1, BLOC], F32)
            sc.activation(invzrow[:], lnz[:], AF.Exp, scale=-1.0)

            # broadcasts [128, BLOC] via K=1 matmul
            zs2b = cst.tile([NN, BLOC], F32)
            c1b = cst.tile([NN, BLOC], F32)
            for row, dst in ((zs2row, zs2b), (c1row, c1b)):
                for c0 in range(0, BLOC, 512):
                    pb_ = pssetup.tile([NN, 512], F32, tag="pe", name=f"bc_{c0}")
                    nc.tensor.matmul(pb_[:], ones128[:], row[:, c0:c0 + 512],
                                     start=True, stop=True)
                    sc.activation(dst[:, c0:c0 + 512], pb_[:], AF.Copy)
            zs4b = cst.tile([NN, BLOC], F32)
            gp.tensor_tensor(zs4b[:], zs2b[:], zs2b[:], OP.mult)

            # node-constant columns
            u4L_c = ncol[:, 0:1]
            nu4L_c = ncol[:, 1:2]
            u4V_c = ncol[:, 2:3]
            nu4V_c = ncol[:, 3:4]
            u2V_c = ncol[:, 4:5]
            wLre_c = wcols[:, 0:1]
            wLim_c = wcols[:, 1:2]
            wVre_c = wcols[:, 2:3]
            wVim_c = wcols[:, 3:4]

            nhln2 = cst.tile([NN, 1], F32)
            v.memset(nhln2[:], -0.5 * LN2)
            c_one = cst.tile([NN, 1], F32)
            v.memset(c_one[:], 1.0)
            c_negk = cst.tile([NN, 1], F32)
            v.memset(c_negk[:], -(1.0 - EPS))
            c_eps = cst.tile([NN, 1], F32)
            v.memset(c_eps[:], EPS)
            c_onep = cst.tile([NN, 1], F32)
            v.memset(c_onep[:], 1.0 + EPS)

            def reduce_sum(wcol, rhs, nm):
                red = pspool.tile([1, BLOC], F32, tag="pr", name=nm)
                for c0 in range(0, BLOC, 512):
                    nc.tensor.matmul(red[0:1, c0:c0 + 512], wcol,
                                     rhs[:, c0:c0 + 512], start=True, stop=True)
                return red

            # ================ L pass ================
            pa_ps = pspool.tile([NN, BLOC], F32, tag="pa", name="paL")
            pb_ps = pspool.tile([NN, BLOC], F32, tag="pb", name="pbL")
            for c0 in range(0, BLOC, 512):
                nc.tensor.matmul(pa_ps[:, c0:c0 + 512], upL[:], rhs_pa[:, c0:c0 + 512],
                                 start=True, stop=True)
            for c0 in range(0, BLOC, 512):
                nc.tensor.matmul(pb_ps[:, c0:c0 + 512], upL[:], rhs_pb[:, c0:c0 + 512],
                                 start=True, stop=True)

            e_a2 = W("a0")
            sc.activation(e_a2[:], pa_ps[:], AF.Exp)
            omz4 = W("a2")
            v.tensor_scalar(omz4[:], zs4b[:], nu4L_c, 1.0, OP.mult, OP.add)
            X = W("a4")
            gp.tensor_tensor(X[:], omz4[:], e_a2[:], OP.mult)
            n2 = W("a6")
            v._custom_dve(_AFFSQ, out=n2[:], in0=c1b[:], s0=u4L_c, s1=EPS,
                          imm2=EPS2)
            rn2 = W("a5")
            v.reciprocal_approx_fast(rn2[:], n2[:])
            t_ = W("a6")
            v.tensor_tensor(t_[:], X[:], rn2[:], OP.mult)
            tDp = W("a4")
            v._custom_dve(_MULAFF, out=tDp[:], in0=t_[:], in1=c1b[:], s0=u4L_c,
                          s1=EPS)
            sgn = W("a8", dt=BF16)
            sc.activation(sgn[:], t_[:], AF.Sign, bias=c_one[:, 0:1], scale=-1.0)
            sqre = W("a5")
            sc.activation(sqre[:], tDp[:], AF.Square, bias=c_negk[:, 0:1], scale=1.0)
            sqim = W("a9")
            sc.activation(sqim[:], t_[:], AF.Square, bias=c_eps[:, 0:1], scale=-EPS)
            r2s = W("a6")
            gp.tensor_tensor(r2s[:], sqre[:], sqim[:], OP.add)
            lnom = W("a7")
            sc.activation(lnom[:], omz4[:], AF.Ln)
            lnr2s = W("a5")
            sc.activation(lnr2s[:], r2s[:], AF.Ln)
            r_ = W("aA", dt=BF16)
            sc.activation(r_[:], lnr2s[:], AF.Exp, scale=0.5)
            absre = W("a2", dt=BF16)
            sc.activation(absre[:], tDp[:], AF.Abs, bias=c_negk[:, 0:1], scale=1.0)
            rlg = W("a3", dt=BF16)
            v.tensor_tensor(rlg[:], absre[:], r_[:], OP.add)
            lnrlg = W("aB")
            sc.activation(lnrlg[:], rlg[:], AF.Ln)
            base = W("a0")
            v.tensor_tensor(base[:], pb_ps[:], lnom[:], OP.subtract)
            base2 = W("a2")
            v.tensor_tensor(base2[:], base[:], lnr2s[:], OP.subtract)
            lnim2 = W("a6")
            sc.activation(lnim2[:], sqim[:], AF.Ln)
            lnglg = W("a5")
            v.tensor_tensor(lnglg[:], base2[:], lnrlg[:], OP.add)
            SS = W("a1", dt=BF16)            # -> becomes igq after swap
            sc.activation(SS[:], lnglg[:], AF.Exp, bias=nhln2[:, 0:1], scale=0.5)
            prt = W("a3")
            gp.tensor_tensor(prt[:], base2[:], lnrlg[:], OP.subtract)
            lngsm = W("a0")
            v.tensor_tensor(lngsm[:], prt[:], lnim2[:], OP.add)
            TTs = W("a7", dt=BF16)           # -> becomes igre after swap
            sc.activation(TTs[:], lngsm[:], AF.Exp, bias=nhln2[:, 0:1], scale=0.5)
            TTs2 = W("a9", dt=BF16)
            v.tensor_copy(TTs2[:], TTs[:])
            m = W("aA", dt=BF16)
            gp.tensor_scalar(m[:], tDp[:], 1.0 - EPS, None, OP.is_ge)
            v.copy_predicated(TTs[:], m[:].bitcast(U16), SS[:])
            v.copy_predicated(SS[:], m[:].bitcast(U16), TTs2[:])
            igqs = W("a2", dt=BF16)
            v.tensor_tensor(igqs[:], SS[:], sgn[:], OP.mult)
            redLre = reduce_sum(wLre_c, TTs, "redLre")
            redLim = reduce_sum(wLim_c, igqs, "redLim")
            outLre = cst.tile([1, BLOC], F32)
            sc.activation(outLre[:], redLre[0:1, :], AF.Copy)
            nc.sync.dma_start(out=out_d[0, :].rearrange("(o n) -> o n", o=1),
                              in_=outLre[:])
            outLim = cst.tile([1, BLOC], F32)
            sc.activation(outLim[:], redLim[0:1, :], AF.Copy)
            nc.sync.dma_start(out=out_d[1, :].rearrange("(o n) -> o n", o=1),
                              in_=outLim[:])

            # ================ V pass ================
            pa2_ps = pspool.tile([NN, BLOC], F32, tag="pa", name="paV")
            pab_ps = pspool.tile([NN, BLOC], F32, tag="pb", name="pabV")
            for c0 in range(0, BLOC, 512):
                nc.tensor.matmul(pa2_ps[:, c0:c0 + 512], upV[:], rhs_pa[:, c0:c0 + 512],
                                 start=True, stop=True)
            for c0 in range(0, BLOC, 512):
                nc.tensor.matmul(pab_ps[:, c0:c0 + 512], upV[:],
                                 rhs_pab[:, c0:c0 + 512], start=True, stop=True)

            e_a2v = W("b0")
            sc.activation(e_a2v[:], pa2_ps[:], AF.Exp)
            omz4v = W("b2")
            v.tensor_scalar(omz4v[:], zs4b[:], nu4V_c, 1.0, OP.mult, OP.add)
            Y = W("b3")
            v.tensor_scalar(Y[:], c1b[:], u4V_c, None, OP.mult)
            Xv = W("b4")
            gp.tensor_tensor(Xv[:], omz4v[:], e_a2v[:], OP.mult)
            n2v = W("b0")
            v._custom_dve(_AFFSQ, out=n2v[:], in0=Xv[:], s0=1.0, s1=EPS,
                          imm2=EPS2)
            rn2v = W("b2")
            v.reciprocal_approx_fast(rn2v[:], n2v[:])
            t2 = W("b0")
            v.tensor_tensor(t2[:], Y[:], rn2v[:], OP.mult)
            t2Xp = W("b3")
            v._custom_dve(_MULAFF, out=t2Xp[:], in0=t2[:], in1=Xv[:], s0=1.0,
                          s1=EPS)
            sqre2 = W("b0")
            sc.activation(sqre2[:], t2Xp[:], AF.Square, bias=c_onep[:, 0:1], scale=-1.0)
            sqim2 = W("b2")
            sc.activation(sqim2[:], t2[:], AF.Square, bias=c_eps[:, 0:1], scale=EPS)
            r2s2 = W("b4")
            gp.tensor_tensor(r2s2[:], sqre2[:], sqim2[:], OP.add)
            lnr2s2 = W("b0")
            sc.activation(lnr2s2[:], r2s2[:], AF.Ln)
            r2v = W("b5", dt=BF16)
            sc.activation(r2v[:], lnr2s2[:], AF.Exp, scale=0.5)
            absre2 = W("b6", dt=BF16)
            sc.activation(absre2[:], t2Xp[:], AF.Abs, bias=c_onep[:, 0:1], scale=-1.0)
            rlg2 = W("b1", dt=BF16)
            v.tensor_tensor(rlg2[:], absre2[:], r2v[:], OP.add)
            lnrlg2 = W("b5")
            sc.activation(lnrlg2[:], rlg2[:], AF.Ln)
            base2v = W("b2")
            v.tensor_tensor(base2v[:], pab_ps[:], lnr2s2[:], OP.subtract)
            lnim2v = W("b6")
            sc.activation(lnim2v[:], sqim2[:], AF.Ln)
            lnglg2 = W("b4")
            v.tensor_tensor(lnglg2[:], base2v[:], lnrlg2[:], OP.add)
            SSv = W("b0")                    # -> becomes M2 after swap
            sc.activation(SSv[:], lnglg2[:], AF.Exp, bias=nhln2[:, 0:1], scale=0.5)
            prt_v = W("b1")
            gp.tensor_tensor(prt_v[:], base2v[:], lnrlg2[:], OP.subtract)
            lngsm2 = W("b4")
            v.tensor_tensor(lngsm2[:], prt_v[:], lnim2v[:], OP.add)
            TTv = W("b2")                    # -> becomes P2 after swap
            sc.activation(TTv[:], lngsm2[:], AF.Exp, bias=nhln2[:, 0:1], scale=0.5)
            TTv2 = W("b5")
            sc.activation(TTv2[:], TTv[:], AF.Copy)
            m2 = W("b1", dt=BF16)
            gp.tensor_scalar(m2[:], t2Xp[:], 1.0 + EPS, None, OP.is_le)
            v.copy_predicated(TTv[:], m2[:].bitcast(U16), SSv[:])
            v.copy_predicated(SSv[:], m2[:].bitcast(U16), TTv2[:])
            P2 = TTv
            M2 = SSv

            zdb = W("b3", dt=BF16)
            v.tensor_scalar(zdb[:], zs2b[:], u2V_c, EPS, OP.mult, OP.add)
            ndn = W("b5")
            v._custom_dve(_AFFSQ, out=ndn[:], in0=zdb[:], s0=1.0, s1=0.0,
                          imm2=EPS2)
            rndr = W("b4")
            v.reciprocal_approx_fast(rndr[:], ndn[:])

            P2m = W("b6", dt=BF16)
            v.tensor_scalar(P2m[:], P2[:], -1.0, None, OP.add)
            M2b = W("b5", dt=BF16)
            sc.activation(M2b[:], M2[:], AF.Copy)
            A12 = W("b1", dt=BF16)
            v.tensor_tensor(A12[:], P2m[:], zdb[:], OP.mult)
            A4 = W("b2", dt=BF16)
            v.scalar_tensor_tensor(A4[:], M2b[:], -EPS, A12[:], OP.mult, OP.add)
            igre = W("b1", dt=BF16)
            v.tensor_tensor(igre[:], A4[:], rndr[:], OP.mult)
            B1 = W("b0", dt=BF16)
            v.tensor_tensor(B1[:], M2b[:], zdb[:], OP.mult)
            B3 = W("b3", dt=BF16)
            v.scalar_tensor_tensor(B3[:], P2m[:], EPS, B1[:], OP.mult, OP.add)
            igim = W("b2", dt=BF16)
            v.tensor_tensor(igim[:], B3[:], rndr[:], OP.mult)
            redVre = reduce_sum(wVre_c, igre, "redVre")
            redVim = reduce_sum(wVim_c, igim, "redVim")

            # ---------------- finals ----------------
            Vr1 = R("f0", nm="Vr1")
            v.tensor_tensor(Vr1[:], redVre[0:1, :], zrow[:], OP.mult)
            outVre = cst.tile([1, BLOC], F32)
            v.scalar_tensor_tensor(outVre[:], invzrow[:], -2.0 * math.pi, Vr1[:],
                                   OP.mult, OP.add)
            nc.sync.dma_start(out=out_d[2, :].rearrange("(o n) -> o n", o=1),
                              in_=outVre[:])
            outVim = cst.tile([1, BLOC], F32)
            v.tensor_tensor(outVim[:], redVim[0:1, :], zrow[:], OP.mult)
            nc.sync.dma_start(out=out_d[3, :].rearrange("(o n) -> o n", o=1),
                              in_=outVim[:])
    return nc


_NC_CACHE = {}


def _restrict_act_tables(nc):
    """Monkeypatch table-set selection to the one set that serves every
    activation this kernel uses (exp/ln/square/sign/abs/copy/identity) so
    the steady state has zero ACT_TABLE_LOADs."""
    import types
    from concourse.hw_specs import get_activation_tables

    def _patched(self):
        tables = [(k, (v if k == "natural_log_exp_and_others" else set()))
                  for k, v in get_activation_tables(self.m.arch).items()]
        bacc._bass_rust.insert_act_table_loads(self, tables)

    nc.insert_act_table_loads = types.MethodType(_patched, nc)


def kernel(a, b, zs):
    a = np.asarray(a, dtype=np.float32)
    b = np.asarray(b, dtype=np.float32)
    zs = np.asarray(zs, dtype=np.float32)
    if "nc" not in _NC_CACHE:
        nc0 = build_nc()
        _restrict_act_tables(nc0)
        nc0.finalize()
        _NC_CACHE["nc"] = nc0
    nc = _NC_CACHE["nc"]
    in_maps = [
        {"a": a, "b": b, "zs": zs[i * BLOC: (i + 1) * BLOC].copy()}
        for i in range(NCORES)
    ]
    res = bass_utils.run_bass_kernel_spmd(nc, in_maps, core_ids=list(range(NCORES)))
    out = np.concatenate([res.results[i]["out"] for i in range(NCORES)], axis=1)
    return out.astype(np.float32)


if __name__ == "__main__":
    rng = np.random.default_rng(0)
    out = kernel(
        rng.standard_normal(5).astype(np.float32),
        rng.standard_normal(5).astype(np.float32),
        (0.02 + 0.975 * rng.random(8192)).astype(np.float32),
    )
    print(out.shape, out.dtype, out[:, :3])
